# revision 7
# baseline (speedup 1.0000x reference)
"""DCNv2 (deformable conv) on 8 TRN2 NeuronCores — v2.

Changes vs v1 baseline:
  - Blend multiplies run in DVE 2x mode: coefficients stored in a
    duplicated-pair layout cf2[..., 2] so the broadcast AP's last dim is
    packed ([1,2]) instead of stride-0 (stride-0 gates 2x off).
  - Blend accumulation is a bf16 pairwise tree (2x adds) instead of an
    fp32 running accumulator (1x adds).
  - Coefficient tensor built in 3 fused ops instead of 81 small muls.
  - ~45% of the 81 blend combos run on the (otherwise idle) GPSIMD/Pool
    engine; DVE and Pool each reduce their own subset, one join at the end.
  - input+fea packed into one [128, ...] tile (fea on partitions 64-127,
    offset-conv weights replicated there) to free SBUF for vv double
    buffering, which overlaps the V/eviction stage of unit i+1 with the
    blend of unit i.
"""

import sys

sys.path.insert(0, "/opt/trn_rl_repo")

import numpy as np
import ml_dtypes

import concourse.bass as bass
import concourse.mybir as mybir
from concourse import tile

f32 = mybir.dt.float32
bf16 = mybir.dt.bfloat16
AF = mybir.ActivationFunctionType

B, C, H, W = 2, 64, 224, 224
BAND = 56  # output rows per core
NCH = 2  # x-chunks
CW = 112  # chunk width
QR = 14  # out rows per quarter-band
QY = 18  # V rows per quarter-band (QR + 4 halo)


def _ap(t, offset_elems, dims):
    base = t[:] if hasattr(t, "tile_id") or not isinstance(t, bass.AP) else t
    return bass.AP(base.tensor, base.offset + offset_elems, [list(d) for d in dims])


def build_nc():
    nc = bass.Bass()
    # comb: partitions 0-63 = input (pad 2), partitions 64-127 = fea (pad 1,
    # padded out to the same 60x228 footprint)
    comb = nc.declare_dram_parameter("comb", [128, 60, 228], bf16, isOutput=False)
    woff = nc.declare_dram_parameter("woff", [128, 9, 27], bf16, isOutput=False)
    wdcn = nc.declare_dram_parameter("wdcn", [64, 9, 64], bf16, isOutput=False)
    boff = nc.declare_dram_parameter("boff", [128, 27], f32, isOutput=False)
    bdcn = nc.declare_dram_parameter("bdcn", [128, 64], f32, isOutput=False)
    outs = [
        nc.declare_dram_parameter(f"out{u}", [14, 112, 64], bf16, isOutput=True)
        for u in range(8)
    ]

    MUL = mybir.AluOpType.mult
    ADD = mybir.AluOpType.add

    with tile.TileContext(nc) as tc:
        # (k, sx) pairs grouped by window shift v = kx + sx; m-index = position
        PAIRS = []  # m -> (k, sx)
        VSTART = {}
        for v in range(5):
            VSTART[v] = len(PAIRS)
            for kx in range(3):
                for sx in range(3):
                    if kx + sx == v:
                        for ky in range(3):
                            PAIRS.append((ky * 3 + kx, sx))
        M_OF = {}
        for m, (k, sx) in enumerate(PAIRS):
            M_OF[(k, sx)] = m

        # 27 fused triples (each covers sy=0..2): (m, ky, k, sx)
        TRIPLES = []
        for kx in range(3):
            for sx in range(3):
                for ky in range(3):
                    k = ky * 3 + kx
                    TRIPLES.append((M_OF[(k, sx)], ky, k, sx))
        DVE_TRIPLES = TRIPLES[:15]
        POOL_TRIPLES = TRIPLES[15:]

        # V eviction groups: 3 evictions/row, 9 m's each (merge v-groups)
        EV_GROUPS = [((0, 1),), ((2,),), ((3, 4),)]

        with (
            tc.tile_pool(name="img", bufs=1) as imgp,
            tc.tile_pool(name="wts", bufs=1) as wtsp,
            tc.tile_pool(name="vv", bufs=2) as vvp,
            tc.tile_pool(name="om", bufs=2) as omp,
            tc.tile_pool(name="coefs", bufs=2) as coefp,
            tc.tile_pool(name="tmp", bufs=1) as tmpp,
            tc.tile_pool(name="scr", bufs=1) as scrp,
            tc.tile_pool(name="tots", bufs=2) as totp,
            tc.tile_pool(name="accs", bufs=2) as accp,
            tc.tile_pool(name="ps_om", bufs=2, space="PSUM") as ps_om,
            tc.tile_pool(name="ps_u", bufs=2, space="PSUM") as ps_u,
        ):
            combs = imgp.tile([128, 60, 228], bf16, tag="combs")
            woff_s = wtsp.tile([128, 9, 27], bf16, tag="woff")
            wdcn_s = wtsp.tile([64, 9, 64], bf16, tag="wdcn")
            boff_s = wtsp.tile([128, 27], f32, tag="boff")
            bdcn_s = wtsp.tile([128, 64], f32, tag="bdcn")
            nc.sync.dma_start(combs[:], comb[:])
            nc.sync.dma_start(woff_s[:], woff[:])
            nc.sync.dma_start(wdcn_s[:], wdcn[:])
            nc.sync.dma_start(boff_s[:], boff[:])
            nc.sync.dma_start(bdcn_s[:], bdcn[:])

            # bias replicated over rows once, in bf16, for packed 2x adds
            bdcn_bf = wtsp.tile([CW, QR, 64], bf16, tag="bdcn_bf", name="bdcn_bf")
            nc.vector.tensor_copy(
                bdcn_bf[:],
                _ap(bdcn_s[:], 0, [[bdcn_s[:].ap[0][0], CW], [0, QR], [1, 64]]),
            )

            # PE warm-up: observe each PE-read DMA semaphore once
            warm = ps_om.tile([1, 1], f32, tag="warm", name="warm")
            nc.tensor.matmul(warm[:], combs[0:64, 0, 0:1], wdcn_s[:, 0, 0:1], start=True, stop=True)
            nc.tensor.matmul(warm[:], wdcn_s[:, 0, 0:1], combs[0:64, 0, 0:1], start=True, stop=True)
            warm2 = ps_om.tile([1, 1], f32, tag="warm", name="warm2")
            nc.tensor.matmul(warm2[:], combs[64:128, 0, 0:1], woff_s[64:128, 0, 0:1], start=True, stop=True)

            state = {}

            def coef_stage(u):
                qb, ch = divmod(u, 2)
                st = {}
                om_t = omp.tile([CW, QR, 27], f32, tag="om", name="om_t")
                for r in range(QR):
                    pom = ps_om.tile([CW, 27], f32, tag="pom", name="pom")
                    for k in range(9):
                        ky, kx = divmod(k, 3)
                        frow = qb * QR + r + ky
                        c0 = ch * CW + kx
                        nc.tensor.matmul(
                            pom[:],
                            combs[64:128, frow, c0 : c0 + CW],
                            woff_s[64:128, k, :],
                            start=(k == 0),
                            stop=(k == 8),
                        )
                    nc.scalar.copy(om_t[:, r, :], pom[:])
                omp0 = om_t[:].ap[0][0]
                nc.vector.tensor_add(
                    _ap(om_t[:], 0, [[omp0, CW], [27, QR], [1, 27]]),
                    _ap(om_t[:], 0, [[omp0, CW], [27, QR], [1, 27]]),
                    _ap(boff_s[:], 0, [[boff_s[:].ap[0][0], CW], [0, QR], [1, 27]]),
                )

                m_in = tmpp.tile([CW, 9, QR], f32, tag="m_in", name="m_in")
                m_t = tmpp.tile([CW, 9, QR], f32, tag="m", name="m_t")
                absf = tmpp.tile([CW, 9, QR], f32, tag="absf", name="absf")
                wy = tmpp.tile([CW, 3, 9, QR], f32, tag="wy", name="wy")
                wx = tmpp.tile([CW, 3, 9, QR], f32, tag="wx", name="wx")
                wym = tmpp.tile([CW, 3, 9, QR], f32, tag="wym", name="wym")
                cf2 = coefp.tile([CW, 9, 9, QR, 2], bf16, tag="cf2", name="cf2")

                nc.vector.tensor_copy(
                    m_in[:], _ap(om_t[:], 18, [[omp0, CW], [1, 9], [27, QR]])
                )
                nc.scalar.activation(m_t[:], m_in[:], AF.Sigmoid)
                for (axis, wt) in ((0, wy), (1, wx)):
                    srcap = _ap(om_t[:], axis, [[omp0, CW], [2, 9], [27, QR]])
                    nc.scalar.activation(wt[:, 0], srcap, AF.Relu, scale=-1.0)
                    nc.scalar.activation(wt[:, 2], srcap, AF.Relu)
                    nc.scalar.activation(absf[:], srcap, AF.Abs)
                    nc.scalar.activation(wt[:, 1], absf[:], AF.Copy, bias=1.0, scale=-1.0)
                for sy in range(3):
                    nc.vector.tensor_mul(wym[:, sy], wy[:, sy], m_t[:])

                wymp = wym[:].ap[0][0]
                wxp = wx[:].ap[0][0]
                cfp = cf2[:].ap[0][0]
                for sy in range(3):
                    for sx in range(3):
                        t = sy * 3 + sx
                        nc.vector.tensor_tensor(
                            _ap(cf2[:], t * 9 * QR * 2,
                                [[cfp, CW], [QR * 2, 9], [2, QR], [1, 2]]),
                            _ap(wym[:], sy * 9 * QR,
                                [[wymp, CW], [QR, 9], [1, QR], [0, 2]]),
                            _ap(wx[:], sx * 9 * QR,
                                [[wxp, CW], [QR, 9], [1, QR], [0, 2]]),
                            MUL,
                        )
                st["cf2"] = cf2
                st["cfp"] = cfp
                state[u] = st

            def v_stage(u):
                qb, ch = divmod(u, 2)
                st = state[u]
                vv = vvp.tile([CW, QY, 27, 64], bf16, tag="vv", name="vv")
                for yp in range(QY):
                    irow = qb * QR + yp
                    for vgrp in ((0, 1), (2,), (3, 4)):
                        mstart = VSTART[vgrp[0]]
                        nmm = sum(
                            (VSTART[v + 1] if v < 4 else 27) - VSTART[v]
                            for v in vgrp
                        )
                        pu = ps_u.tile([CW, 9, 64], f32, tag="pu", name="pu")
                        jj = 0
                        for v in vgrp:
                            nv = (VSTART[v + 1] if v < 4 else 27) - VSTART[v]
                            lhsT = combs[0:64, irow, ch * CW + v : ch * CW + v + CW]
                            for j in range(nv):
                                k, sx = PAIRS[VSTART[v] + j]
                                nc.tensor.matmul(
                                    pu[:, jj, :],
                                    lhsT,
                                    wdcn_s[:, k, :],
                                    start=True,
                                    stop=True,
                                )
                                jj += 1
                        nc.scalar.copy(
                            _ap(vv[:], (yp * 27 + mstart) * 64,
                                [[vv[:].ap[0][0], CW], [1, nmm * 64]]),
                            pu[:, 0:nmm, :],
                        )
                st["vv"] = vv

            def blend_stage(u):
                st = state[u]
                vv = st["vv"]
                cf2 = st["cf2"]
                cfp = st["cfp"]
                vvp0 = vv[:].ap[0][0]
                acc = accp.tile([CW, QR, 64], bf16, tag="acc", name="acc")

                def emit_blend(eng, triples, scrt, etot):
                    for i, (m, ky, k, sx) in enumerate(triples):
                        for sy in range(3):
                            in0 = _ap(
                                vv[:],
                                ((ky + sy) * 27 + m) * 64,
                                [[vvp0, CW], [27 * 64, QR], [1, 64]],
                            )
                            in1 = _ap(
                                cf2[:],
                                ((sy * 3 + sx) * 9 + k) * QR * 2,
                                [[cfp, CW], [2, QR], [0, 32], [1, 2]],
                            )
                            eng.tensor_tensor(scrt[:, sy], in0, in1, MUL)
                        eng.tensor_tensor(scrt[:, 0], scrt[:, 0], scrt[:, 1], ADD)
                        if i == 0:
                            eng.tensor_tensor(etot[:], scrt[:, 0], scrt[:, 2], ADD)
                        else:
                            eng.tensor_tensor(scrt[:, 0], scrt[:, 0], scrt[:, 2], ADD)
                            eng.tensor_tensor(etot[:], etot[:], scrt[:, 0], ADD)

                scr_d = scrp.tile([CW, 3, QR, 64], bf16, tag="scr_d", name="scr_d")
                scr_p = scrp.tile([CW, 3, QR, 64], bf16, tag="scr_p", name="scr_p")
                tot_d = totp.tile([CW, QR, 64], bf16, tag="tot_d", name="tot_d")
                tot_p = totp.tile([CW, QR, 64], bf16, tag="tot_p", name="tot_p")
                emit_blend(nc.vector, DVE_TRIPLES, scr_d, tot_d)
                emit_blend(nc.gpsimd, POOL_TRIPLES, scr_p, tot_p)

                nc.vector.tensor_tensor(acc[:], tot_d[:], tot_p[:], ADD)
                nc.vector.tensor_tensor(acc[:], acc[:], bdcn_bf[:], ADD)
                dst = _ap(
                    outs[u][:],
                    0,
                    [[64, CW], [CW * 64, QR], [1, 64]],
                )
                accsrc = _ap(acc[:], 0, [[acc[:].ap[0][0], CW], [64, QR], [1, 64]])
                nc.sync.dma_start(dst, accsrc)
                del state[u]

            # software pipeline: coef+V of unit i run ahead of blend of i-1
            for u in range(8):
                coef_stage(u)
                v_stage(u)
                if u >= 1:
                    blend_stage(u - 1)
            blend_stage(7)

    # ---- post-pass: strip vacuous same-engine waits (ISA slot limits)
    eng_prefix = {
        mybir.EngineType.PE: "PE_",
        mybir.EngineType.DVE: "DVE_",
        mybir.EngineType.Activation: "Activation_",
        mybir.EngineType.Pool: "Pool_",
        mybir.EngineType.SP: "SP_",
    }
    for bb in nc.main_func.blocks:
        for ins in bb.instructions:
            pref = eng_prefix.get(getattr(ins, "engine", None))
            if pref and ins.sync_info and ins.sync_info.on_wait:
                ow = ins.sync_info.on_wait
                kept = [w for w in ow if not (w.ant_name or "").startswith(pref)]
                if len(kept) != len(ow):
                    ins.sync_info.on_wait = kept
    for bb in nc.main_func.blocks:
        for ins in bb.instructions:
            if type(ins).__name__ == "InstDMACopy" and ins.sync_info and ins.sync_info.on_wait:
                onames = [a.bass_ap.tensor.name for a in ins.outs if hasattr(a, "bass_ap")]
                if any(n.startswith("out") for n in onames):
                    kept = [w for w in ins.sync_info.on_wait if not (w.ant_name or "").startswith("DMAHW")]
                    if len(kept) != len(ins.sync_info.on_wait):
                        ins.sync_info.on_wait = kept
    import copy as _copy
    # Per-engine Drain prototypes (Tile emits at least one per engine).
    drain_proto = {}
    for bb in nc.main_func.blocks:
        for ins in bb.instructions:
            if type(ins).__name__ == "InstDrain":
                drain_proto.setdefault(ins.engine, ins)
    # Any instruction carrying >1 sync wait exceeds some engines' ISA slot
    # budget (TT structs hold 1). Hoist extras into a chain of same-engine
    # drains placed immediately before it (in-order engines make this safe).
    for bb in nc.main_func.blocks:
        i = 0
        while i < len(bb.instructions):
            ins = bb.instructions[i]
            ow = list(ins.sync_info.on_wait or []) if ins.sync_info else []
            if len(ow) > 1 and getattr(ins, "engine", None) in drain_proto:
                proto = drain_proto[ins.engine]
                ins.sync_info.on_wait = ow[-1:]
                for ci, w in enumerate(ow[:-1]):
                    d2 = _copy.deepcopy(proto)
                    d2.name = f"{ins.name}-w{ci}"
                    si = _copy.deepcopy(ins.sync_info)
                    si.on_wait = [w]
                    si.on_update = []
                    d2.sync_info = si
                    bb.instructions.insert(i, d2)
                    i += 1
            i += 1
    return nc


_cached = {}


def prepare_in_maps(input, fea, w_off, b_off, w_dcn, b_dcn):
    input = np.asarray(input, dtype=np.float32)
    fea = np.asarray(fea, dtype=np.float32)
    w_off = np.asarray(w_off, dtype=np.float32)
    b_off = np.asarray(b_off, dtype=np.float32)
    w_dcn = np.asarray(w_dcn, dtype=np.float32)
    b_dcn = np.asarray(b_dcn, dtype=np.float32)

    woff9 = np.zeros((64, 9, 27), np.float32)
    wdcn9 = np.zeros((64, 9, 64), np.float32)
    for ky in range(3):
        for kx in range(3):
            k = ky * 3 + kx
            woff9[:, k, :] = w_off[:, :, ky, kx].T
            wdcn9[:, k, :] = w_dcn[:, :, ky, kx].T
    woff2 = np.zeros((128, 9, 27), np.float32)
    woff2[64:] = woff9  # offset conv runs on partitions 64-127
    woff2 = woff2.astype(ml_dtypes.bfloat16)
    wdcn9 = wdcn9.astype(ml_dtypes.bfloat16)
    boff_e = np.ascontiguousarray(np.broadcast_to(b_off[None, :], (128, 27))).astype(np.float32)
    bdcn_e = np.ascontiguousarray(np.broadcast_to(b_dcn[None, :], (128, 64))).astype(np.float32)

    in_maps = []
    for core in range(8):
        b, band = divmod(core, 4)
        r0 = band * BAND
        cb = np.zeros((128, 60, 228), np.float32)
        ys, ye = max(r0 - 2, 0), min(r0 + 58, H)
        cb[0:64, ys - (r0 - 2) : ye - (r0 - 2), 2:226] = input[b, :, ys:ye, :]
        ys2, ye2 = max(r0 - 1, 0), min(r0 + 57, H)
        cb[64:128, ys2 - (r0 - 1) : ye2 - (r0 - 1), 1:225] = fea[b, :, ys2:ye2, :]
        in_maps.append(
            dict(
                comb=cb.astype(ml_dtypes.bfloat16),
                woff=woff2,
                wdcn=wdcn9,
                boff=boff_e,
                bdcn=bdcn_e,
            )
        )
    return in_maps


def kernel(input, fea, w_off, b_off, w_dcn, b_dcn):
    in_maps = prepare_in_maps(input, fea, w_off, b_off, w_dcn, b_dcn)
    if "nc" not in _cached:
        _cached["nc"] = build_nc()
    from concourse.bass_utils import run_bass_kernel_spmd

    res = run_bass_kernel_spmd(_cached["nc"], in_maps, core_ids=list(range(8)))
    globals()["last_results"] = res
    out = np.zeros((2, 64, H, W), np.float32)
    for core in range(8):
        b, band = divmod(core, 4)
        blk = np.zeros((56, 224, 64), np.float32)
        for u in range(8):
            qb, ch = divmod(u, 2)
            blk[qb * 14 : (qb + 1) * 14, ch * 112 : (ch + 1) * 112, :] = np.asarray(
                res.results[core][f"out{u}"]
            ).astype(np.float32).reshape(14, 112, 64)
        out[b, :, band * BAND : (band + 1) * BAND, :] = blk.transpose(2, 0, 1)
    return out
